# revision 11
# baseline (speedup 1.0000x reference)
"""Multi-head attention Trainium2 kernel (Bass/Tile), data-parallel over batch.

Problem shapes (hardcoded): x [8, 1024, 1024] fp32, 16 heads x 64 dim,
shared per-head projections Wq/Wk/Wv [64, 64], output proj Wo [1024, 1024].

Reference math (note quirks):
  xh = x reshaped to [h, b, m, d]
  Q/K/V = xh @ W{q,k,v}.T + b
  scores = einsum('hbmd,hbnd->hbmn', K, Q) / sqrt(1024)   (K @ Q^T!)
  A = softmax(scores, axis=-1)
  out = (A @ V) transposed (0,1,3,2) then .reshape(b, m, D) @ Wo.T + bo

Per-core plan (core b handles batch b, no collectives):
  - host prepares xT = x[b].T, blockdiag lhsT weights for 2-head packed
    projections, WoT = Wo.T
  - QT/KT/VT [64*16, m] computed via blockdiag [128,128] matmuls
  - per head: S_T[n, m] = QT.T @ KT (scores transposed); even/odd head
    matmuls interleaved (disjoint PE row groups run concurrently); exp on
    ACT with scale 1/32 (softmax max-subtraction skipped; scores are O(1))
  - U[65, m] = [V | ones].T @ expS  -> row 64 = softmax denominator
  - PE-transpose U -> [m, 65], normalize cols by reciprocal of col 64 -> P.T
  - Y rows for the pair's heads = P.T chunk.T @ WoT, interleaved per pair
    (bo added on host); host scatters Y rows (j = h*64+d) into full output
Matmul dtype configurable: "f32r" (fp22 multiply, ~2.5e-4 rel err) or
"f16" (fp16 multiply, faster weight loads, ~1e-3 rel err).
"""

import os

import numpy as np

B = 8
M = 1024
D = 1024
NT = 8  # 128-row tiles in M / D

DTYPE_MODE = os.environ.get("KERNEL_DTYPE", "f32r")

_compiled = {}


def _build(mode):
    import concourse.bacc as bacc
    import concourse.mybir as mybir
    import concourse.tile as tile
    from concourse.masks import make_identity

    f32 = mybir.dt.float32
    mdt = mybir.dt.float32r if mode == "f32r" else mybir.dt.float16
    Exp = mybir.ActivationFunctionType.Exp

    nc = bacc.Bacc("TRN2", target_bir_lowering=False, debug=False, num_devices=B)

    xT_ap = nc.dram_tensor("xT", [D, M], mdt, kind="ExternalInput").ap()
    woT_ap = nc.dram_tensor("woT", [D, D], mdt, kind="ExternalInput").ap()
    wq_ap = nc.dram_tensor("wq", [128, 128], mdt, kind="ExternalInput").ap()
    wk_ap = nc.dram_tensor("wk", [128, 128], mdt, kind="ExternalInput").ap()
    wv_ap = nc.dram_tensor("wv", [128, 128], mdt, kind="ExternalInput").ap()
    bias_ap = nc.dram_tensor("bias", [128, 3], f32, kind="ExternalInput").ap()
    y_ap = nc.dram_tensor("y", [D, M], f32, kind="ExternalOutput").ap()

    with tile.TileContext(nc) as tc:
        with (
            tc.tile_pool(name="persist", bufs=1) as persist,
            tc.tile_pool(name="qkv", bufs=2) as qkv_pool,
            tc.tile_pool(name="vnat", bufs=2) as vnat_pool,
            tc.tile_pool(name="exps", bufs=(4 if mode == "f16" else 2)) as exps_pool,
            tc.tile_pool(name="usb", bufs=3) as usb_pool,
            tc.tile_pool(name="ysb", bufs=2) as ysb_pool,
            tc.tile_pool(name="rec", bufs=4) as rec_pool,
            tc.tile_pool(name="ps", bufs=1, space="PSUM") as ps_pool,
        ):
            # ---- persistent tiles + loads ----
            xT_all = persist.tile([128, NT * M], mdt)  # tile t at cols t*M
            woT_all = persist.tile([128, NT * D], mdt)
            PT_all = persist.tile([128, NT * D], mdt)  # [m-local, mt*D + h*64+d]
            wq_sb = persist.tile([128, 128], mdt)
            wk_sb = persist.tile([128, 128], mdt)
            wv_sb = persist.tile([128, 128], mdt)
            bias_sb = persist.tile([128, 3], f32)
            identity = persist.tile([128, 128], f32)

            with nc.named_scope("loads"):
                nc.sync.dma_start(wq_sb[:], wq_ap[:])
                nc.sync.dma_start(wk_sb[:], wk_ap[:])
                nc.sync.dma_start(wv_sb[:], wv_ap[:])
                nc.sync.dma_start(bias_sb[:], bias_ap[:])
                for t in range(NT):
                    nc.sync.dma_start(
                        xT_all[:, t * M : (t + 1) * M], xT_ap[t * 128 : (t + 1) * 128, :]
                    )
                for t in range(NT):
                    nc.sync.dma_start(
                        woT_all[:, t * D : (t + 1) * D],
                        woT_ap[t * 128 : (t + 1) * 128, :],
                    )
                make_identity(nc, identity[:])

            # ---- per head-pair: QKV, attention, then that pair's slice of
            # the output projection (keeps PE dense and the tail short) ----
            for t in range(8):
                with nc.named_scope(f"qkv_p{t}"):
                    qT = qkv_pool.tile([128, M], mdt, tag="qT")
                    kT = qkv_pool.tile([128, M], mdt, tag="kT")
                    vT = qkv_pool.tile([128, M], f32, tag="vT")
                    for pi, (w_sb, dest) in enumerate(
                        ((wq_sb, qT), (wk_sb, kT), (wv_sb, vT))
                    ):
                        ps = ps_pool.tile([128, 1024], f32, tag="w", bufs=1)
                        for mh in range(2):
                            nc.tensor.matmul(
                                ps[:, mh * 512 : (mh + 1) * 512],
                                w_sb[:],
                                xT_all[:, t * M + mh * 512 : t * M + (mh + 1) * 512],
                                start=True,
                                stop=True,
                            )
                        nc.vector.tensor_scalar_add(
                            dest[:], ps[:], bias_sb[:, pi : pi + 1]
                        )

                    # V natural (both heads) + ones cols:
                    # per nt block of 130: [even 64 | 1 | odd 64 | 1]
                    v_nat = vnat_pool.tile([128, NT * 130], mdt, tag="vn")
                    pst = ps_pool.tile([128, 1024], f32, tag="w", bufs=1)
                    for nt in range(NT):
                        nc.tensor.transpose(
                            pst[:, nt * 128 : (nt + 1) * 128],
                            vT[:, nt * 128 : (nt + 1) * 128],
                            identity[:],
                        )
                    for nt in range(NT):
                        o = nt * 130
                        nc.vector.tensor_copy(
                            v_nat[:, o : o + 64], pst[:, nt * 128 : nt * 128 + 64]
                        )
                        nc.vector.tensor_copy(
                            v_nat[:, o + 65 : o + 129],
                            pst[:, nt * 128 + 64 : (nt + 1) * 128],
                        )
                        ones_cast = f32 if mode == "f32r" else mdt
                        nc.gpsimd.memset(
                            v_nat[:, o + 64 : o + 65].bitcast(ones_cast), 1.0
                        )
                        nc.gpsimd.memset(
                            v_nat[:, o + 129 : o + 130].bitcast(ones_cast), 1.0
                        )

                # attention for both heads of the pair; even/odd score
                # matmuls adjacent -> concurrent on disjoint PE row groups
                u_sbs = []
                for hh in range(2):
                    u_sbs.append(usb_pool.tile([65, M], f32, tag="u", name="u_sb"))
                for mh in range(2):
                    with nc.named_scope(f"attn_p{t}_m{mh}"):
                        expS = [
                            exps_pool.tile([128, NT * 512], mdt, tag="es", name="expS_e"),
                            exps_pool.tile([128, NT * 512], mdt, tag="es", name="expS_o"),
                        ]
                        for ntp in range(4):
                            psS = [
                                ps_pool.tile([128, 1024], f32, tag="s", bufs=2, name="psS_e"),
                                ps_pool.tile([128, 1024], f32, tag="s", bufs=2, name="psS_o"),
                            ]
                            for sub in range(2):
                                nt = 2 * ntp + sub
                                for hh in range(2):
                                    part = hh * 64
                                    nc.tensor.matmul(
                                        psS[hh][:, sub * 512 : (sub + 1) * 512],
                                        qT[
                                            part : part + 64,
                                            nt * 128 : (nt + 1) * 128,
                                        ],
                                        kT[
                                            part : part + 64,
                                            mh * 512 : (mh + 1) * 512,
                                        ],
                                        start=True,
                                        stop=True,
                                    )
                            for hh in range(2):
                                nc.scalar.activation(
                                    expS[hh][:, ntp * 1024 : (ntp + 1) * 1024],
                                    psS[hh][:],
                                    Exp,
                                    scale=1.0 / 32.0,
                                )
                        for hh in range(2):
                            psU = ps_pool.tile([65, 512], f32, tag="u", bufs=2)
                            for nt in range(NT):
                                o = nt * 130 + hh * 65
                                nc.tensor.matmul(
                                    psU[:],
                                    v_nat[:, o : o + 65],
                                    expS[hh][:, nt * 512 : (nt + 1) * 512],
                                    start=(nt == 0),
                                    stop=(nt == NT - 1),
                                )
                            nc.vector.tensor_copy(
                                u_sbs[hh][:, mh * 512 : (mh + 1) * 512], psU[:]
                            )
                for hh in range(2):
                    h = 2 * t + hh
                    u_sb = u_sbs[hh]
                    with nc.named_scope(f"norm_h{h}"):
                        pstU = ps_pool.tile([128, 1024], f32, tag="w", bufs=1)
                        for mt in range(NT):
                            nc.tensor.transpose(
                                pstU[:, mt * 128 : mt * 128 + 65],
                                u_sb[:, mt * 128 : (mt + 1) * 128],
                                identity[:65, :65],
                            )
                        for mt in range(NT):
                            rec = rec_pool.tile([128, 1], f32, tag="r")
                            nc.vector.reciprocal(
                                rec[:], pstU[:, mt * 128 + 64 : mt * 128 + 65]
                            )
                            nc.vector.tensor_scalar_mul(
                                PT_all[:, mt * D + h * 64 : mt * D + h * 64 + 64],
                                pstU[:, mt * 128 : mt * 128 + 64],
                                rec[:],
                            )

                # output projection rows for this pair (j = 128t .. 128t+127)
                with nc.named_scope(f"final_p{t}"):
                    psY = ps_pool.tile([128, 1024], f32, tag="w", bufs=1)
                    for dh in range(2):
                        for mt in range(NT):
                            nc.tensor.matmul(
                                psY[:, dh * 512 : (dh + 1) * 512],
                                PT_all[:, mt * D + t * 128 : mt * D + (t + 1) * 128],
                                woT_all[:, mt * D + dh * 512 : mt * D + (dh + 1) * 512],
                                start=(mt == 0),
                                stop=(mt == NT - 1),
                            )
                    y_sb = ysb_pool.tile([128, 1024], f32, tag="y")
                    nc.vector.tensor_copy(y_sb[:], psY[:])
                    nc.sync.dma_start(y_ap[t * 128 : (t + 1) * 128, :], y_sb[:])

    nc.compile()
    return nc


def _get_compiled(mode):
    if mode not in _compiled:
        _compiled[mode] = _build(mode)
    return _compiled[mode]


def _prep_inputs(mode, x, Wq, bq, Wk, bk, Wv, bv, Wo, bo):
    np_mdt = np.float32 if mode == "f32r" else np.float16

    def blockdiag_lhsT(W):
        out = np.zeros((128, 128), np.float32)
        out[:64, :64] = W.T
        out[64:, 64:] = W.T
        return out.astype(np_mdt)

    wq_bd = blockdiag_lhsT(Wq)
    wk_bd = blockdiag_lhsT(Wk)
    wv_bd = blockdiag_lhsT(Wv)
    bias = np.stack(
        [np.concatenate([b, b]) for b in (bq, bk, bv)], axis=1
    ).astype(np.float32)  # [128, 3]
    woT = np.ascontiguousarray(Wo.T).astype(np_mdt)
    xT = np.ascontiguousarray(np.transpose(x, (0, 2, 1))).astype(np_mdt)  # [B, D, M]
    in_maps = [
        {
            "xT": xT[b],
            "woT": woT,
            "wq": wq_bd,
            "wk": wk_bd,
            "wv": wv_bd,
            "bias": bias,
        }
        for b in range(B)
    ]
    return in_maps


def run(inputs, trace=False, trace_kwargs=None, mode=DTYPE_MODE):
    """Run on HW; returns (full_output, BassKernelResults)."""
    from concourse.bass_utils import run_bass_kernel_spmd

    inputs = {k: np.asarray(v) for k, v in inputs.items()}
    nc = _get_compiled(mode)
    in_maps = _prep_inputs(
        mode,
        inputs["x"],
        inputs["Wq"], inputs["bq"],
        inputs["Wk"], inputs["bk"],
        inputs["Wv"], inputs["bv"],
        inputs["Wo"], inputs["bo"],
    )
    kw = dict(trace_kwargs or {})
    res = run_bass_kernel_spmd(nc, in_maps, list(range(B)), trace=trace, **kw)
    out = np.empty((B, M, D), np.float32)
    out5 = out.reshape(B, 2, 8, 64, D)  # [bo, s, b, d, Do]
    for b in range(B):
        Y = res.results[b]["y"]  # [1024(j=h*64+d), 1024(Do)]
        out5[:, :, b] = Y.reshape(8, 2, 64, D)
    out += np.asarray(inputs["bo"], np.float32)[None, None, :]
    return out, res


def kernel(**inputs):
    out, _ = run(inputs)
    return out


# revision 13
# speedup vs baseline: 1.4166x; 1.4166x over previous
"""Multi-head attention Trainium2 kernel (Bass/Tile), data-parallel over batch.

Problem shapes (hardcoded): x [8, 1024, 1024] fp32, 16 heads x 64 dim,
shared per-head projections Wq/Wk/Wv [64, 64], output proj Wo [1024, 1024].

Reference math (note quirks):
  xh = x reshaped to [h, b, m, d]
  Q/K/V = xh @ W{q,k,v}.T + b
  scores = einsum('hbmd,hbnd->hbmn', K, Q) / sqrt(1024)   (K @ Q^T!)
  A = softmax(scores, axis=-1)
  out = (A @ V) transposed (0,1,3,2) then .reshape(b, m, D) @ Wo.T + bo

Per-core plan (core b handles batch b, no collectives):
  - host prepares xT = x[b].T, blockdiag lhsT weights for 2-head packed
    projections, WoT = Wo.T
  - QT/KT/VT [64*16, m] computed via blockdiag [128,128] matmuls
  - per head: S_T[n, m] = QT.T @ KT (scores transposed); even/odd head
    matmuls interleaved (disjoint PE row groups run concurrently); exp on
    ACT with scale 1/32 (softmax max-subtraction skipped; scores are O(1))
  - U[65, m] = [V | ones].T @ expS  -> row 64 = softmax denominator
  - PE-transpose U -> [m, 65], normalize cols by reciprocal of col 64 -> P.T
  - Y rows for the pair's heads = P.T chunk.T @ WoT, interleaved per pair
    (bo added on host); host scatters Y rows (j = h*64+d) into full output
Matmul dtype configurable: "f32r" (fp22 multiply, ~2.5e-4 rel err) or
"f16" (fp16 multiply, faster weight loads, ~1e-3 rel err).
"""

import os

import numpy as np

B = 8
M = 1024
D = 1024
NT = 8  # 128-row tiles in M / D

DTYPE_MODE = os.environ.get("KERNEL_DTYPE", "f32r")

_compiled = {}


def _build(mode):
    import concourse.bacc as bacc
    import concourse.mybir as mybir
    import concourse.tile as tile
    from concourse.masks import make_identity

    f32 = mybir.dt.float32
    mdt = mybir.dt.float32r if mode == "f32r" else mybir.dt.float16
    Exp = mybir.ActivationFunctionType.Exp

    nc = bacc.Bacc("TRN2", target_bir_lowering=False, debug=False, num_devices=B)

    xT_ap = nc.dram_tensor("xT", [D, M], mdt, kind="ExternalInput").ap()
    woT_ap = nc.dram_tensor("woT", [D, D], mdt, kind="ExternalInput").ap()
    wq_ap = nc.dram_tensor("wq", [128, 128], mdt, kind="ExternalInput").ap()
    wk_ap = nc.dram_tensor("wk", [128, 128], mdt, kind="ExternalInput").ap()
    wv_ap = nc.dram_tensor("wv", [128, 128], mdt, kind="ExternalInput").ap()
    bias_ap = nc.dram_tensor("bias", [128, 3], f32, kind="ExternalInput").ap()
    y_ap = nc.dram_tensor("y", [D, M], f32, kind="ExternalOutput").ap()

    with tile.TileContext(nc) as tc:
        with (
            tc.tile_pool(name="persist", bufs=1) as persist,
            tc.tile_pool(name="qkv", bufs=2) as qkv_pool,
            tc.tile_pool(name="vnat", bufs=2) as vnat_pool,
            tc.tile_pool(name="exps", bufs=(4 if mode == "f16" else 2)) as exps_pool,
            tc.tile_pool(name="usb", bufs=3) as usb_pool,
            tc.tile_pool(name="ysb", bufs=2) as ysb_pool,
            tc.tile_pool(name="rec", bufs=4) as rec_pool,
            tc.tile_pool(name="ps", bufs=1, space="PSUM") as ps_pool,
        ):
            # ---- persistent tiles + loads ----
            xT_all = persist.tile([128, NT * M], mdt)  # tile t at cols t*M
            woT_all = persist.tile([128, NT * D], mdt)
            PT_all = persist.tile([128, NT * D], mdt)  # [m-local, mt*D + h*64+d]
            wq_sb = persist.tile([128, 128], mdt)
            wk_sb = persist.tile([128, 128], mdt)
            wv_sb = persist.tile([128, 128], mdt)
            bias_sb = persist.tile([128, 3], f32)
            identity = persist.tile([128, 128], f32)

            with nc.named_scope("loads"):
                nc.sync.dma_start(wq_sb[:], wq_ap[:])
                nc.sync.dma_start(wk_sb[:], wk_ap[:])
                nc.sync.dma_start(wv_sb[:], wv_ap[:])
                nc.sync.dma_start(bias_sb[:], bias_ap[:])
                for t in range(NT):
                    nc.sync.dma_start(
                        xT_all[:, t * M : (t + 1) * M], xT_ap[t * 128 : (t + 1) * 128, :]
                    )
                for t in range(NT):
                    nc.sync.dma_start(
                        woT_all[:, t * D : (t + 1) * D],
                        woT_ap[t * 128 : (t + 1) * 128, :],
                    )
                make_identity(nc, identity[:])

            # ---- per head-pair: QKV, attention, then that pair's slice of
            # the output projection (keeps PE dense and the tail short) ----
            for t in range(8):
                with nc.named_scope(f"qkv_p{t}"):
                    qT = qkv_pool.tile([128, M], mdt, tag="qT")
                    kT = qkv_pool.tile([128, M], mdt, tag="kT")
                    vT = qkv_pool.tile([128, M], f32, tag="vT")
                    for pi, (w_sb, dest) in enumerate(
                        ((wq_sb, qT), (wk_sb, kT), (wv_sb, vT))
                    ):
                        for mh in range(2):
                            ps = ps_pool.tile([128, 512], f32, tag="w", bufs=3)
                            nc.tensor.matmul(
                                ps[:],
                                w_sb[:],
                                xT_all[:, t * M + mh * 512 : t * M + (mh + 1) * 512],
                                start=True,
                                stop=True,
                            )
                            nc.vector.tensor_scalar_add(
                                dest[:, mh * 512 : (mh + 1) * 512],
                                ps[:],
                                bias_sb[:, pi : pi + 1],
                            )

                    # V natural (both heads) + ones cols:
                    # per nt block of 130: [even 64 | 1 | odd 64 | 1]
                    v_nat = vnat_pool.tile([128, NT * 130], mdt, tag="vn")
                    for g in range(2):
                        pst = ps_pool.tile([128, 512], f32, tag="w", bufs=3)
                        for j in range(4):
                            nt = 4 * g + j
                            nc.tensor.transpose(
                                pst[:, j * 128 : (j + 1) * 128],
                                vT[:, nt * 128 : (nt + 1) * 128],
                                identity[:],
                            )
                        for j in range(4):
                            nt = 4 * g + j
                            o = nt * 130
                            nc.vector.tensor_copy(
                                v_nat[:, o : o + 64], pst[:, j * 128 : j * 128 + 64]
                            )
                            nc.vector.tensor_copy(
                                v_nat[:, o + 65 : o + 129],
                                pst[:, j * 128 + 64 : (j + 1) * 128],
                            )
                            ones_cast = f32 if mode == "f32r" else mdt
                            nc.gpsimd.memset(
                                v_nat[:, o + 64 : o + 65].bitcast(ones_cast), 1.0
                            )
                            nc.gpsimd.memset(
                                v_nat[:, o + 129 : o + 130].bitcast(ones_cast), 1.0
                            )

                # attention for both heads of the pair; even/odd score
                # matmuls adjacent -> concurrent on disjoint PE row groups
                u_sbs = []
                for hh in range(2):
                    u_sbs.append(usb_pool.tile([65, M], f32, tag="u", name="u_sb"))
                for mh in range(2):
                    with nc.named_scope(f"attn_p{t}_m{mh}"):
                        expS = [
                            exps_pool.tile([128, NT * 512], mdt, tag="es", name="expS_e"),
                            exps_pool.tile([128, NT * 512], mdt, tag="es", name="expS_o"),
                        ]
                        for ntp in range(4):
                            psS = [
                                ps_pool.tile([128, 1024], f32, tag="s", bufs=2, name="psS_e"),
                                ps_pool.tile([128, 1024], f32, tag="s", bufs=2, name="psS_o"),
                            ]
                            for sub in range(2):
                                nt = 2 * ntp + sub
                                for hh in range(2):
                                    part = hh * 64
                                    nc.tensor.matmul(
                                        psS[hh][:, sub * 512 : (sub + 1) * 512],
                                        qT[
                                            part : part + 64,
                                            nt * 128 : (nt + 1) * 128,
                                        ],
                                        kT[
                                            part : part + 64,
                                            mh * 512 : (mh + 1) * 512,
                                        ],
                                        start=True,
                                        stop=True,
                                    )
                            for hh in range(2):
                                nc.scalar.activation(
                                    expS[hh][:, ntp * 1024 : (ntp + 1) * 1024],
                                    psS[hh][:],
                                    Exp,
                                    scale=1.0 / 32.0,
                                )
                        for hh in range(2):
                            psU = ps_pool.tile([65, 512], f32, tag="u", bufs=1)
                            for nt in range(NT):
                                o = nt * 130 + hh * 65
                                nc.tensor.matmul(
                                    psU[:],
                                    v_nat[:, o : o + 65],
                                    expS[hh][:, nt * 512 : (nt + 1) * 512],
                                    start=(nt == 0),
                                    stop=(nt == NT - 1),
                                )
                            nc.vector.tensor_copy(
                                u_sbs[hh][:, mh * 512 : (mh + 1) * 512], psU[:]
                            )
                for hh in range(2):
                    h = 2 * t + hh
                    u_sb = u_sbs[hh]
                    with nc.named_scope(f"norm_h{h}"):
                        for g in range(2):
                            pstU = ps_pool.tile([128, 512], f32, tag="w", bufs=3)
                            for j in range(4):
                                mt = 4 * g + j
                                nc.tensor.transpose(
                                    pstU[:, j * 128 : j * 128 + 65],
                                    u_sb[:, mt * 128 : (mt + 1) * 128],
                                    identity[:65, :65],
                                )
                            for j in range(4):
                                mt = 4 * g + j
                                rec = rec_pool.tile([128, 1], f32, tag="r")
                                nc.vector.reciprocal(
                                    rec[:], pstU[:, j * 128 + 64 : j * 128 + 65]
                                )
                                nc.vector.tensor_scalar_mul(
                                    PT_all[:, mt * D + h * 64 : mt * D + h * 64 + 64],
                                    pstU[:, j * 128 : j * 128 + 64],
                                    rec[:],
                                )

                # output projection rows for this pair (j = 128t .. 128t+127)
                with nc.named_scope(f"final_p{t}"):
                    y_sb = ysb_pool.tile([128, 1024], f32, tag="y")
                    for dh in range(2):
                        psY = ps_pool.tile([128, 512], f32, tag="w", bufs=3)
                        for mt in range(NT):
                            nc.tensor.matmul(
                                psY[:],
                                PT_all[:, mt * D + t * 128 : mt * D + (t + 1) * 128],
                                woT_all[:, mt * D + dh * 512 : mt * D + (dh + 1) * 512],
                                start=(mt == 0),
                                stop=(mt == NT - 1),
                            )
                        nc.vector.tensor_copy(
                            y_sb[:, dh * 512 : (dh + 1) * 512], psY[:]
                        )
                    nc.sync.dma_start(y_ap[t * 128 : (t + 1) * 128, :], y_sb[:])

    nc.compile()
    return nc


def _get_compiled(mode):
    if mode not in _compiled:
        _compiled[mode] = _build(mode)
    return _compiled[mode]


def _prep_inputs(mode, x, Wq, bq, Wk, bk, Wv, bv, Wo, bo):
    np_mdt = np.float32 if mode == "f32r" else np.float16

    def blockdiag_lhsT(W):
        out = np.zeros((128, 128), np.float32)
        out[:64, :64] = W.T
        out[64:, 64:] = W.T
        return out.astype(np_mdt)

    wq_bd = blockdiag_lhsT(Wq)
    wk_bd = blockdiag_lhsT(Wk)
    wv_bd = blockdiag_lhsT(Wv)
    bias = np.stack(
        [np.concatenate([b, b]) for b in (bq, bk, bv)], axis=1
    ).astype(np.float32)  # [128, 3]
    woT = np.ascontiguousarray(Wo.T).astype(np_mdt)
    xT = np.ascontiguousarray(np.transpose(x, (0, 2, 1))).astype(np_mdt)  # [B, D, M]
    in_maps = [
        {
            "xT": xT[b],
            "woT": woT,
            "wq": wq_bd,
            "wk": wk_bd,
            "wv": wv_bd,
            "bias": bias,
        }
        for b in range(B)
    ]
    return in_maps


def run(inputs, trace=False, trace_kwargs=None, mode=DTYPE_MODE):
    """Run on HW; returns (full_output, BassKernelResults)."""
    from concourse.bass_utils import run_bass_kernel_spmd

    inputs = {k: np.asarray(v) for k, v in inputs.items()}
    nc = _get_compiled(mode)
    in_maps = _prep_inputs(
        mode,
        inputs["x"],
        inputs["Wq"], inputs["bq"],
        inputs["Wk"], inputs["bk"],
        inputs["Wv"], inputs["bv"],
        inputs["Wo"], inputs["bo"],
    )
    kw = dict(trace_kwargs or {})
    res = run_bass_kernel_spmd(nc, in_maps, list(range(B)), trace=trace, **kw)
    out = np.empty((B, M, D), np.float32)
    out5 = out.reshape(B, 2, 8, 64, D)  # [bo, s, b, d, Do]
    for b in range(B):
        Y = res.results[b]["y"]  # [1024(j=h*64+d), 1024(Do)]
        out5[:, :, b] = Y.reshape(8, 2, 64, D)
    out += np.asarray(inputs["bo"], np.float32)[None, None, :]
    return out, res


def kernel(**inputs):
    out, _ = run(inputs)
    return out
